# revision 5
# baseline (speedup 1.0000x reference)
"""Trainium2 Bass kernel for the CPUMamba problem (B=2, T=2048, D=1024,
d_inner=2048, d_state=16, dt_rank=64, d_conv=4) on 8 NeuronCores.

Sharding: 2 batch groups x 4 d_inner slices (E_c=512 per core).
On-chip layout: [channels -> partitions, time -> free].  Host pre-transposes
x and all weights so the kernel does no on-chip transposes.

Scan: lane = (8 e-channels x 16 states), dt/u broadcast e->(e,n) by one-hot
selector matmuls on PE (consumed straight from PSUM); h = exp(A*dt)*h +
(dt*xc)*B via DVE tensor_tensor_scan along the free (time) axis;
y = sum_n C*h via one-hot placer matmuls accumulating in PSUM.
x_dbl is AllReduced over each batch group's 4 e-shards; the output is
computed as out_proj partials and ReduceScattered so each core owns a
d-slice ([D/4, T], d-major; host transposes back).
"""
import sys

if '/opt/trn_rl_repo' not in sys.path:
    sys.path.insert(0, '/opt/trn_rl_repo')

import numpy as np

# ---------------- problem constants (hardcoded) ----------------
D_MODEL = 1024
D_STATE = 16
D_CONV = 4
D_INNER = 2048
DT_RANK = 64
B_SZ, T_SEQ = 2, 2048
N_GROUPS = 2          # batch groups
E_C = 512             # d_inner slice per core
GC = D_INNER // E_C   # cores per group (4)
N_CORES = N_GROUPS * GC


# ---------------- kernel builder ----------------
def build_mamba_nc(D, E, E_c, T, N=16, R=64, DCONV=4, n_groups=2,
                   z_engine="gpsimd", gate_engine="gpsimd", act_impl="hw"):
    import concourse.bass as bass
    import concourse.mybir as mybir
    from concourse.tile import TileContext

    F32 = mybir.dt.float32
    AF = mybir.ActivationFunctionType
    OP = mybir.AluOpType

    Gc = E // E_c                 # cores per batch group
    K = D // 128                  # k-tiles for in_proj contraction
    M = E_c // 128                # e-tiles per core
    G = E_c // 8                  # scan tiles per core
    MO = D // 128                 # out d-tiles
    TC = min(512, T)              # matmul free-dim chunk
    NT = T // TC
    XD = R + 2 * N                # x_dbl channels (96)
    DS = D // Gc                  # output d-slice per core
    assert D % 128 == 0 and E_c % 128 == 0 and T % TC == 0
    assert R <= 128 and XD <= 128 and E_c % 8 == 0 and D % Gc == 0

    nc = bass.Bass()

    xT = nc.declare_dram_parameter("xT", [D, T], F32, isOutput=False)
    w_xzT = nc.declare_dram_parameter("w_xzT", [D, 2 * E_c], F32, isOutput=False)
    xprojT = nc.declare_dram_parameter("xprojT", [E_c, XD], F32, isOutput=False)
    dtprojT = nc.declare_dram_parameter("dtprojT", [R, E_c], F32, isOutput=False)
    opT = nc.declare_dram_parameter("opT", [E_c, D], F32, isOutput=False)
    conv_w = nc.declare_dram_parameter("conv_w", [128, M * DCONV], F32, isOutput=False)
    conv_b = nc.declare_dram_parameter("conv_b", [128, M], F32, isOutput=False)
    dtb = nc.declare_dram_parameter("dtb", [128, M], F32, isOutput=False)
    Dp = nc.declare_dram_parameter("Dp", [128, M], F32, isOutput=False)
    Alane = nc.declare_dram_parameter("Alane", [128, G], F32, isOutput=False)
    sel_bcast = nc.declare_dram_parameter("sel_bcast", [128, 16 * 128], F32, isOutput=False)
    sel_place = nc.declare_dram_parameter("sel_place", [128, 16 * 128], F32, isOutput=False)
    sel_rep16 = nc.declare_dram_parameter("sel_rep16", [N, 128], F32, isOutput=False)

    out_c = nc.declare_dram_parameter("out_c", [DS, T], F32, isOutput=True)

    xdbl_loc = nc.dram_tensor("xdbl_loc", [XD, T], F32)
    xdbl_red = nc.dram_tensor("xdbl_red", [XD, T], F32)
    s_dram = nc.dram_tensor("s_dram", [E_c, T], F32)
    out_loc = nc.dram_tensor("out_loc", [D, T], F32)
    out_red = nc.dram_tensor("out_red", [DS, T], F32)

    groups = [[g * Gc + i for i in range(Gc)] for g in range(n_groups)]

    def silu_into(pool, out_ap, in_ap, bias_ap=None):
        if act_impl == "hw":
            if bias_ap is None:
                nc.scalar.activation(out_ap, in_ap, AF.Silu)
            else:
                nc.scalar.activation(out_ap, in_ap, AF.Silu, bias=bias_ap)
        else:  # sim fallback: silu(v) = v * sigmoid(v), v = in + bias
            P_, F_ = in_ap.shape[0], in_ap.shape[1]
            v = pool.tile([P_, F_], F32, tag="siluv", name="siluv")
            nc.vector.tensor_scalar(v[:], in_ap,
                                    bias_ap if bias_ap is not None else 0.0,
                                    None, OP.add)
            sg = pool.tile([P_, F_], F32, tag="silusg", name="silusg")
            nc.scalar.activation(sg[:], v[:], AF.Sigmoid)
            nc.vector.tensor_tensor(out_ap, v[:], sg[:], OP.mult)

    def softplus_into(pool, out_ap, in_ap, bias_ap):
        # softplus(v) = ln(1 + exp(v)); the walrus lower_act pass has no
        # table set for the Softplus enum, so decompose (Exp and Ln share
        # the natural_log_exp_and_others set).
        P_, F_ = in_ap.shape[0], in_ap.shape[1]
        e = pool.tile([P_, F_], F32, tag="spe", name="spe", bufs=2)
        nc.scalar.activation(e[:], in_ap, AF.Exp, bias=bias_ap)
        nc.vector.tensor_scalar(e[:], e[:], 1.0, None, OP.add)
        nc.scalar.activation(out_ap, e[:], AF.Ln)

    with TileContext(nc) as tc:
        with tc.tile_pool(name="persist", bufs=1) as pp:
            # ---------- small persistent params ----------
            xp_sb = pp.tile([128, M * XD], F32)
            nc.sync.dma_start(out=xp_sb[:], in_=xprojT.ap().rearrange("(m p) e -> p m e", p=128))
            dtp_sb = pp.tile([R, E_c], F32)
            nc.sync.dma_start(out=dtp_sb[:], in_=dtprojT[:, :])
            cw_sb = pp.tile([128, M * DCONV], F32)
            nc.sync.dma_start(out=cw_sb[:], in_=conv_w[:, :])
            cb_sb = pp.tile([128, M], F32)
            nc.sync.dma_start(out=cb_sb[:], in_=conv_b[:, :])
            dtb_sb = pp.tile([128, M], F32)
            nc.sync.dma_start(out=dtb_sb[:], in_=dtb[:, :])
            dp_sb = pp.tile([128, M], F32)
            nc.sync.dma_start(out=dp_sb[:], in_=Dp[:, :])
            al_sb = pp.tile([128, G], F32)
            nc.sync.dma_start(out=al_sb[:], in_=Alane[:, :])
            sb_sb = pp.tile([128, 16 * 128], F32)
            nc.sync.dma_start(out=sb_sb[:], in_=sel_bcast[:, :])
            spl_sb = pp.tile([128, 16 * 128], F32)
            nc.sync.dma_start(out=spl_sb[:], in_=sel_place[:, :])
            sr_sb = pp.tile([N, 128], F32)
            nc.sync.dma_start(out=sr_sb[:], in_=sel_rep16[:, :])

            xc_sb = [pp.tile([128, T], F32, tag=f"xc{m}", name=f"xc{m}") for m in range(M)]
            dt_sb = [pp.tile([128, T], F32, tag=f"dt{m}", name=f"dt{m}") for m in range(M)]

            # ---------- phase A: in_proj + fused conv ----------
            # xi is consumed chunk-by-chunk through a rolling [128, TC+3]
            # window per m; silu(z) streams to DRAM (s_dram).
            with (
                tc.tile_pool(name="pA", bufs=1) as pa,
                tc.tile_pool(name="pAw", bufs=2) as paw,
                tc.tile_pool(name="pAx", bufs=K + 2) as pax,
                tc.tile_pool(name="psA", bufs=4, space="PSUM") as psA,
            ):
                wxz = pa.tile([128, K * 2 * E_c], F32)
                nc.sync.dma_start(
                    out=wxz[:], in_=w_xzT.ap().rearrange("(k p) e -> p k e", p=128))
                win = [pa.tile([128, TC + DCONV - 1], F32, tag=f"win{m}", name=f"win{m}")
                       for m in range(M)]
                for m in range(M):
                    nc.vector.memset(win[m][:, 0:DCONV - 1], 0)

                for t in range(NT):
                    xtc = []
                    for k in range(K):
                        xk = pax.tile([128, TC], F32, tag="xt", name="xt")
                        nc.sync.dma_start(
                            out=xk[:], in_=xT[k * 128:(k + 1) * 128, t * TC:(t + 1) * TC])
                        xtc.append(xk)
                    for m in range(2 * M):
                        ps = psA.tile([128, TC], F32, tag="a", name="ps_a")
                        for k in range(K):
                            nc.tensor.matmul(
                                ps[:],
                                wxz[:, k * 2 * E_c + m * 128: k * 2 * E_c + (m + 1) * 128],
                                xtc[k][:],
                                start=(k == 0), stop=(k == K - 1),
                            )
                        if m < M:  # xi chunk into rolling window
                            nc.scalar.copy(
                                out=win[m][:, DCONV - 1:DCONV - 1 + TC], in_=ps[:])
                            # conv on this chunk
                            acc = paw.tile([128, TC], F32, tag="conv", name="conv_acc")
                            nc.vector.tensor_scalar(
                                acc[:], win[m][:, 0:TC],
                                cw_sb[:, m * DCONV: m * DCONV + 1], None, OP.mult)
                            for kk in range(1, DCONV):
                                accn = paw.tile([128, TC], F32, tag="conv", name="conv_accn")
                                nc.vector.scalar_tensor_tensor(
                                    out=accn[:], in0=win[m][:, kk:kk + TC],
                                    scalar=cw_sb[:, m * DCONV + kk: m * DCONV + kk + 1],
                                    in1=acc[:], op0=OP.mult, op1=OP.add)
                                acc = accn
                            silu_into(paw, xc_sb[m][:, t * TC:(t + 1) * TC], acc[:],
                                      cb_sb[:, m:m + 1])
                            if t < NT - 1:  # roll last 3 cols to the front
                                nc.vector.tensor_copy(
                                    out=win[m][:, 0:DCONV - 1],
                                    in_=win[m][:, TC:TC + DCONV - 1])
                        else:      # z part -> silu -> DRAM
                            sz = paw.tile([128, TC], F32, tag="sz", name="sz")
                            silu_into(paw, sz[:], ps[:])
                            nc.sync.dma_start(
                                out=s_dram[(m - M) * 128:(m - M + 1) * 128,
                                           t * TC:(t + 1) * TC],
                                in_=sz[:])

            # ---------- x_proj partials + all-reduce ----------
            with (
                tc.tile_pool(name="pXd", bufs=1) as pxd,
                tc.tile_pool(name="psB", bufs=2, space="PSUM") as psB,
            ):
                xdbl_sb = pxd.tile([XD, T], F32)
                for t in range(NT):
                    ps = psB.tile([XD, TC], F32, tag="b", name="ps_b")
                    for m in range(M):
                        nc.tensor.matmul(
                            ps[:],
                            xp_sb[:, m * XD:(m + 1) * XD],
                            xc_sb[m][:, t * TC:(t + 1) * TC],
                            start=(m == 0), stop=(m == M - 1),
                        )
                    nc.scalar.copy(out=xdbl_sb[:, t * TC:(t + 1) * TC], in_=ps[:])
                nc.sync.dma_start(out=xdbl_loc[:, :], in_=xdbl_sb[:])

            nc.gpsimd.collective_compute(
                "AllReduce", OP.add, replica_groups=groups,
                ins=[xdbl_loc.ap().opt()], outs=[xdbl_red.ap().opt()])

            # ---------- phase C/D: dt_proj -> dt ; B_rep/C_rep ----------
            with tc.tile_pool(name="pBC", bufs=1) as pbc:
                brep_sb = pbc.tile([128, T], F32)
                crep_sb = pbc.tile([128, T], F32)
                with (
                    tc.tile_pool(name="pCD", bufs=1) as pcd,
                    tc.tile_pool(name="psC", bufs=2, space="PSUM") as psC,
                ):
                    dtlo_sb = pcd.tile([R, T], F32)
                    b_sb = pcd.tile([N, T], F32)
                    c_sb = pcd.tile([N, T], F32)
                    nc.sync.dma_start(out=dtlo_sb[:], in_=xdbl_red[0:R, :])
                    nc.sync.dma_start(out=b_sb[:], in_=xdbl_red[R:R + N, :])
                    nc.sync.dma_start(out=c_sb[:], in_=xdbl_red[R + N:XD, :])
                    for m in range(M):
                        for t in range(NT):
                            ps = psC.tile([128, TC], F32, tag='c', name='ps_c')
                            nc.tensor.matmul(
                                ps[:], dtp_sb[:, m * 128:(m + 1) * 128],
                                dtlo_sb[:, t * TC:(t + 1) * TC],
                                start=True, stop=True)
                            softplus_into(pcd, dt_sb[m][:, t * TC:(t + 1) * TC], ps[:],
                                          dtb_sb[:, m:m + 1])
                    for t in range(NT):
                        psb_ = psC.tile([128, TC], F32, tag='c', name='psb_c')
                        nc.tensor.matmul(psb_[:], sr_sb[:],
                                         b_sb[:, t * TC:(t + 1) * TC],
                                         start=True, stop=True)
                        nc.scalar.copy(out=brep_sb[:, t * TC:(t + 1) * TC], in_=psb_[:])
                        psc_ = psC.tile([128, TC], F32, tag='c', name='psc_c')
                        nc.tensor.matmul(psc_[:], sr_sb[:],
                                         c_sb[:, t * TC:(t + 1) * TC],
                                         start=True, stop=True)
                        nc.scalar.copy(out=crep_sb[:, t * TC:(t + 1) * TC], in_=psc_[:])

                # ---------- phase E: scan ----------
                z_eng = getattr(nc, z_engine)
                gate_eng = getattr(nc, gate_engine)
                yd_list = []
                pyd_cm = tc.tile_pool(name="pyd", bufs=M)
                pyd = pyd_cm.__enter__()
                with (
                    tc.tile_pool(name="pScan", bufs=2) as psc_sb,
                    tc.tile_pool(name="psDT", bufs=2, space="PSUM") as psDT,
                    tc.tile_pool(name="psU", bufs=2, space="PSUM") as psU,
                    tc.tile_pool(name="psY", bufs=1, space="PSUM") as psY,
                ):
                    for m in range(M):
                        y_ps = psY.tile([128, T], F32, tag="y", name="y_ps")
                        u_m = psc_sb.tile([128, T], F32, tag="u", name="u_m")
                        nc.vector.tensor_tensor(u_m[:], dt_sb[m][:], xc_sb[m][:], OP.mult)
                        for jj in range(16):
                            g = m * 16 + jj
                            a_t = psc_sb.tile([128, T], F32, tag="a", name="a_t")
                            dbx_t = psc_sb.tile([128, T], F32, tag="dbx", name="dbx_t")
                            for t in range(NT):
                                tcs = slice(t * TC, (t + 1) * TC)
                                dtr = psDT.tile([128, TC], F32, tag="dt", name="dtr")
                                nc.tensor.matmul(dtr[:], sb_sb[:, jj * 128:(jj + 1) * 128],
                                                 dt_sb[m][:, tcs], start=True, stop=True)
                                nc.scalar.activation(a_t[:, tcs], dtr[:], AF.Exp,
                                                     scale=al_sb[:, g:g + 1])
                                ur = psU.tile([128, TC], F32, tag="u", name="ur")
                                nc.tensor.matmul(ur[:], sb_sb[:, jj * 128:(jj + 1) * 128],
                                                 u_m[:, tcs], start=True, stop=True)
                                nc.vector.tensor_tensor(dbx_t[:, tcs], ur[:],
                                                        brep_sb[:, tcs], OP.mult)
                            # h over a_t (in-place), z over h (in-place)
                            nc.vector.tensor_tensor_scan(a_t[:], a_t[:], dbx_t[:], 0.0,
                                                         OP.mult, OP.add)
                            z_eng.tensor_tensor(a_t[:], a_t[:], crep_sb[:], OP.mult)
                            for t in range(NT):
                                tcs = slice(t * TC, (t + 1) * TC)
                                nc.tensor.matmul(y_ps[:, tcs],
                                                 spl_sb[:, jj * 128:(jj + 1) * 128],
                                                 a_t[:, tcs],
                                                 start=(jj == 0), stop=(jj == 15))
                        s_m = psc_sb.tile([128, T], F32, tag="s", name="s_m")
                        nc.sync.dma_start(out=s_m[:],
                                          in_=s_dram[m * 128:(m + 1) * 128, :])
                        yd = pyd.tile([128, T], F32, tag="yd", name="yd")
                        nc.vector.scalar_tensor_tensor(
                            out=yd[:], in0=xc_sb[m][:], scalar=dp_sb[:, m:m + 1],
                            in1=y_ps[:], op0=OP.mult, op1=OP.add)
                        gate_eng.tensor_tensor(yd[:], yd[:], s_m[:], OP.mult)
                        yd_list.append(yd)

                # ---------- phase F: out_proj partials + reduce-scatter ----------
                with (
                    tc.tile_pool(name="pF", bufs=1) as pf,
                    tc.tile_pool(name="pFe", bufs=3) as pfe,
                    tc.tile_pool(name="psF", bufs=2, space="PSUM") as psF,
                ):
                    opT_sb = pf.tile([128, M * D], F32)
                    nc.sync.dma_start(
                        out=opT_sb[:],
                        in_=opT.ap().rearrange("(m p) d -> p m d", p=128))
                    for mo in range(MO):
                        for t in range(NT):
                            tcs = slice(t * TC, (t + 1) * TC)
                            ps = psF.tile([128, TC], F32, tag="f", name="ps_f")
                            for k in range(M):
                                nc.tensor.matmul(
                                    ps[:],
                                    opT_sb[:, k * D + mo * 128: k * D + (mo + 1) * 128],
                                    yd_list[k][:, tcs],
                                    start=(k == 0), stop=(k == M - 1))
                            ev = pfe.tile([128, TC], F32, tag="ev", name="ev")
                            nc.scalar.copy(out=ev[:], in_=ps[:])
                            nc.sync.dma_start(
                                out=out_loc[mo * 128:(mo + 1) * 128, tcs], in_=ev[:])
                pyd_cm.__exit__(None, None, None)

            nc.gpsimd.collective_compute(
                "ReduceScatter", OP.add, replica_groups=groups,
                ins=[out_loc.ap().opt()], outs=[out_red.ap().opt()])
            nc.gpsimd.dma_start(out=out_c[:, :], in_=out_red[:, :])

    return nc


# ---------------- walrus wait-limit workaround ----------------
def split_excess_waits(nc, max_waits=1):
    """This walrus build rejects instructions with >1 sync-wait command.
    Move excess waits onto NoOps inserted before the instruction on the
    same engine (same-engine stream order keeps semantics identical)."""
    import concourse.mybir as mybir
    n_split = 0
    for fn in nc.m.functions:
        for blk in fn.blocks:
            insts = list(blk.instructions)
            out = []
            changed = False
            for inst in insts:
                si = getattr(inst, "sync_info", None)
                waits = list(si.on_wait) if (si is not None and si.on_wait) else []
                if len(waits) > max_waits:
                    chunks = [waits[i:i + max_waits]
                              for i in range(0, len(waits), max_waits)]
                    for j, ch in enumerate(chunks[:-1]):
                        nop = mybir.InstNoOp(
                            name=f"{inst.name}-wsplit{j}", ins=[], outs=[])
                        nop.engine = inst.engine
                        nop.sync_info = mybir.SyncInfo(on_wait=ch, on_update=[])
                        out.append(nop)
                    si.on_wait = chunks[-1]
                    n_split += 1
                    changed = True
                out.append(inst)
            if changed:
                blk.instructions.clear()
                for i in out:
                    blk.instructions.append(i)
    return n_split


# ---------------- host-side prep ----------------
def _make_selectors(N=16):
    sel_bcast = np.zeros((128, 16 * 128), np.float32)
    sel_place = np.zeros((128, 16 * 128), np.float32)
    for j in range(16):
        for l in range(128):
            e = j * 8 + l // 16
            sel_bcast[e, j * 128 + l] = 1.0
            sel_place[l, j * 128 + e] = 1.0
    sel_rep16 = np.zeros((N, 128), np.float32)
    for l in range(128):
        sel_rep16[l % N, l] = 1.0
    return sel_bcast, sel_place, sel_rep16


def make_core_inputs(inputs, D, E, E_c, T, N=16, R=64, DCONV=4, n_groups=2):
    Gc = E // E_c
    M = E_c // 128
    G = E_c // 8
    x = np.asarray(inputs["x"], np.float32)
    in_proj_w = np.asarray(inputs["in_proj_w"], np.float32)
    conv_w = np.asarray(inputs["conv_w"], np.float32)
    conv_b = np.asarray(inputs["conv_b"], np.float32)
    x_proj_w = np.asarray(inputs["x_proj_w"], np.float32)
    dt_proj_w = np.asarray(inputs["dt_proj_w"], np.float32)
    dt_proj_b = np.asarray(inputs["dt_proj_b"], np.float32)
    A_log = np.asarray(inputs["A_log"], np.float32)
    Dp = np.asarray(inputs["Dp"], np.float32)
    out_proj_w = np.asarray(inputs["out_proj_w"], np.float32)

    A = -np.exp(A_log)
    opT_full = np.ascontiguousarray(out_proj_w.T)
    sel_bcast, sel_place, sel_rep16 = _make_selectors(N)

    maps = []
    for c in range(n_groups * Gc):
        b, ec = c // Gc, c % Gc
        es = ec * E_c
        w_xz = np.concatenate(
            [in_proj_w[es:es + E_c], in_proj_w[E + es:E + es + E_c]], axis=0)
        eidx = np.arange(E_c) + es
        lane_e = np.arange(128) // 16
        lane_n = np.arange(128) % 16
        g_idx = np.arange(G)
        Alane = A[(es + g_idx[None, :] * 8 + lane_e[:, None]), lane_n[:, None]]
        pm = np.arange(128)[:, None] + 128 * np.arange(M)[None, :]
        maps.append({
            "xT": np.ascontiguousarray(x[b].T),
            "w_xzT": np.ascontiguousarray(w_xz.T),
            "xprojT": np.ascontiguousarray(x_proj_w[:, es:es + E_c].T),
            "dtprojT": np.ascontiguousarray(dt_proj_w[es:es + E_c].T),
            "opT": np.ascontiguousarray(opT_full[es:es + E_c]),
            "conv_w": np.ascontiguousarray(
                conv_w[eidx].reshape(M, 128, DCONV).transpose(1, 0, 2)
                .reshape(128, M * DCONV)),
            "conv_b": np.ascontiguousarray(conv_b[eidx[pm]]),
            "dtb": np.ascontiguousarray(dt_proj_b[eidx[pm]]),
            "Dp": np.ascontiguousarray(Dp[eidx[pm]]),
            "Alane": np.ascontiguousarray(Alane.astype(np.float32)),
            "sel_bcast": sel_bcast,
            "sel_place": sel_place,
            "sel_rep16": sel_rep16,
        })
    return maps


def assemble_output(core_outs, D, E, E_c, T, B, n_groups=2):
    Gc = E // E_c
    DS = D // Gc
    out = np.empty((B, T, D), np.float32)
    for c in range(n_groups * Gc):
        b, ec = c // Gc, c % Gc
        out[b, :, ec * DS:(ec + 1) * DS] = core_outs[c].T
    return out


# ---------------- cached PJRT runner ----------------
_RUNNER = None


class _Runner:
    """Builds the Bass module once and keeps a reusable jitted shard_map
    callable (mirrors concourse.bass2jax.run_bass_via_pjrt, but cached so
    repeated calls don't recompile)."""

    def __init__(self):
        import jax
        import jax.numpy as jnp  # noqa: F401
        from jax.experimental.shard_map import shard_map
        from jax.sharding import Mesh, PartitionSpec
        import concourse.mybir as mybir
        from concourse import bass2jax

        self.nc = build_mamba_nc(D_MODEL, D_INNER, E_C, T_SEQ, N=D_STATE,
                                 R=DT_RANK, DCONV=D_CONV, n_groups=N_GROUPS)
        split_excess_waits(self.nc)

        bass2jax.install_neuronx_cc_hook()
        nc = self.nc
        assert nc.dbg_addr is None
        partition_name = (nc.partition_id_tensor.name
                          if nc.partition_id_tensor else None)

        in_names, out_names, out_avals, zero_outs = [], [], [], []
        for alloc in nc.m.functions[0].allocations:
            if not isinstance(alloc, mybir.MemoryLocationSet):
                continue
            name = alloc.memorylocations[0].name
            if alloc.kind == "ExternalInput":
                if name != partition_name:
                    in_names.append(name)
            elif alloc.kind == "ExternalOutput":
                shape = tuple(alloc.tensor_shape)
                dtype = mybir.dt.np(alloc.dtype)
                out_names.append(name)
                out_avals.append(jax.core.ShapedArray(shape, dtype))
                zero_outs.append(np.zeros(shape, dtype))
        n_params = len(in_names)
        n_outs = len(out_avals)
        all_names = in_names + out_names
        if partition_name is not None:
            all_names = all_names + [partition_name]
        donate = tuple(range(n_params, n_params + n_outs))
        self.in_names = in_names
        self.out_names = out_names
        self.zero_outs = zero_outs
        self.n_cores = N_CORES

        def _body(*args):
            operands = list(args)
            if partition_name is not None:
                operands.append(bass2jax.partition_id_tensor())
            outs = bass2jax._bass_exec_p.bind(
                *operands,
                out_avals=tuple(out_avals),
                in_names=tuple(all_names),
                out_names=tuple(out_names),
                lowering_input_output_aliases=(),
                sim_require_finite=True,
                sim_require_nnan=True,
                nc=nc,
            )
            return tuple(outs)

        devices = jax.devices()[:N_CORES]
        assert len(devices) == N_CORES
        mesh = Mesh(np.asarray(devices), ("core",))
        in_specs = (PartitionSpec("core"),) * (n_params + n_outs)
        out_specs = (PartitionSpec("core"),) * n_outs
        self._fn = jax.jit(
            shard_map(_body, mesh=mesh, in_specs=in_specs,
                      out_specs=out_specs, check_rep=False),
            donate_argnums=donate, keep_unused=True)
        self._mesh = mesh
        self._jax = jax

    def put_inputs(self, in_maps):
        """Pre-stage concatenated inputs on the device mesh (axis-0 sharded).
        Returns a handle reusable across run_prestaged calls."""
        import jax
        from jax.sharding import NamedSharding, PartitionSpec
        sh = NamedSharding(self._mesh, PartitionSpec("core"))
        concat_in = [
            np.concatenate([np.asarray(in_maps[c][n]) for c in range(self.n_cores)],
                           axis=0)
            for n in self.in_names
        ]
        arrs = [jax.device_put(a, sh) for a in concat_in]
        for a in arrs:
            a.block_until_ready()
        return arrs

    def fresh_zero_outs(self):
        import jax
        from jax.sharding import NamedSharding, PartitionSpec
        sh = NamedSharding(self._mesh, PartitionSpec("core"))
        zs = [jax.device_put(np.concatenate([z] * self.n_cores, axis=0), sh)
              for z in self.zero_outs]
        for z in zs:
            z.block_until_ready()
        return zs

    def run_prestaged(self, dev_in, dev_zeros):
        """Execute with device-resident args; blocks until done; leaves
        outputs on device. Returns the raw jax output tuple."""
        outs = self._fn(*dev_in, *dev_zeros)
        for o in outs:
            o.block_until_ready()
        return outs

    def __call__(self, in_maps):
        concat_in = [
            np.concatenate([np.asarray(in_maps[c][n]) for c in range(self.n_cores)],
                           axis=0)
            for n in self.in_names
        ]
        concat_zero = [
            np.concatenate([z] * self.n_cores, axis=0) for z in self.zero_outs
        ]
        outs = self._fn(*concat_in, *concat_zero)
        results = []
        for c in range(self.n_cores):
            r = {}
            for i, n in enumerate(self.out_names):
                arr = np.asarray(outs[i])
                per = arr.shape[0] // self.n_cores
                r[n] = arr[c * per:(c + 1) * per]
            results.append(r)
        return results


def _get_runner():
    global _RUNNER
    if _RUNNER is None:
        _RUNNER = _Runner()
    return _RUNNER


def kernel(**inputs):
    runner = _get_runner()
    maps = make_core_inputs(inputs, D_MODEL, D_INNER, E_C, T_SEQ, N=D_STATE,
                            R=DT_RANK, DCONV=D_CONV, n_groups=N_GROUPS)
    outs = runner(maps)
    return assemble_output([o["out_c"] for o in outs], D_MODEL, D_INNER, E_C,
                           T_SEQ, B_SZ, n_groups=N_GROUPS)


# revision 7
# speedup vs baseline: 59.7834x; 59.7834x over previous
"""Trainium2 Bass kernel for the CPUMamba problem (B=2, T=2048, D=1024,
d_inner=2048, d_state=16, dt_rank=64, d_conv=4) on 8 NeuronCores.

Sharding: 2 batch groups x 4 d_inner slices (E_c=512 per core).
On-chip layout: [channels -> partitions, time -> free].  Host pre-transposes
x and all weights so the kernel does no on-chip transposes.

Scan: lane = (8 e-channels x 16 states), dt/u broadcast e->(e,n) by one-hot
selector matmuls on PE (consumed straight from PSUM); h = exp(A*dt)*h +
(dt*xc)*B via DVE tensor_tensor_scan along the free (time) axis;
y = sum_n C*h via one-hot placer matmuls accumulating in PSUM.
x_dbl is AllReduced over each batch group's 4 e-shards; the output is
computed as out_proj partials and ReduceScattered so each core owns a
d-slice ([D/4, T], d-major; host transposes back).
"""
import sys

if '/opt/trn_rl_repo' not in sys.path:
    sys.path.insert(0, '/opt/trn_rl_repo')

import numpy as np

# ---------------- problem constants (hardcoded) ----------------
D_MODEL = 1024
D_STATE = 16
D_CONV = 4
D_INNER = 2048
DT_RANK = 64
B_SZ, T_SEQ = 2, 2048
N_GROUPS = 2          # batch groups
E_C = 512             # d_inner slice per core
GC = D_INNER // E_C   # cores per group (4)
N_CORES = N_GROUPS * GC


# ---------------- kernel builder ----------------
def build_mamba_nc(D, E, E_c, T, N=16, R=64, DCONV=4, n_groups=2,
                   z_engine="gpsimd", gate_engine="gpsimd", act_impl="hw",
                   reps=1, dbg_scan_as_mult=False, dbg_no_collectives=False):
    import concourse.bass as bass
    import concourse.mybir as mybir
    from concourse.tile import TileContext

    F32 = mybir.dt.float32
    AF = mybir.ActivationFunctionType
    OP = mybir.AluOpType

    Gc = E // E_c                 # cores per batch group
    K = D // 128                  # k-tiles for in_proj contraction
    M = E_c // 128                # e-tiles per core
    G = E_c // 8                  # scan tiles per core
    MO = D // 128                 # out d-tiles
    TC = min(512, T)              # matmul free-dim chunk
    NT = T // TC
    XD = R + 2 * N                # x_dbl channels (96)
    DS = D // Gc                  # output d-slice per core
    assert D % 128 == 0 and E_c % 128 == 0 and T % TC == 0
    assert R <= 128 and XD <= 128 and E_c % 8 == 0 and D % Gc == 0

    nc = bass.Bass()

    xT = nc.declare_dram_parameter("xT", [D, T], F32, isOutput=False)
    w_xzT = nc.declare_dram_parameter("w_xzT", [D, 2 * E_c], F32, isOutput=False)
    xprojT = nc.declare_dram_parameter("xprojT", [E_c, XD], F32, isOutput=False)
    dtprojT = nc.declare_dram_parameter("dtprojT", [R, E_c], F32, isOutput=False)
    opT = nc.declare_dram_parameter("opT", [E_c, D], F32, isOutput=False)
    conv_w = nc.declare_dram_parameter("conv_w", [128, M * DCONV], F32, isOutput=False)
    conv_b = nc.declare_dram_parameter("conv_b", [128, M], F32, isOutput=False)
    dtb = nc.declare_dram_parameter("dtb", [128, M], F32, isOutput=False)
    Dp = nc.declare_dram_parameter("Dp", [128, M], F32, isOutput=False)
    Alane = nc.declare_dram_parameter("Alane", [128, G], F32, isOutput=False)
    sel_bcast = nc.declare_dram_parameter("sel_bcast", [128, 16 * 128], F32, isOutput=False)
    sel_place = nc.declare_dram_parameter("sel_place", [128, 16 * 128], F32, isOutput=False)
    sel_rep16 = nc.declare_dram_parameter("sel_rep16", [N, 128], F32, isOutput=False)

    out_c = nc.declare_dram_parameter("out_c", [DS, T], F32, isOutput=True)

    xdbl_loc = nc.dram_tensor("xdbl_loc", [XD, T], F32)
    xdbl_red = nc.dram_tensor("xdbl_red", [XD, T], F32)
    s_dram = nc.dram_tensor("s_dram", [E_c, T], F32)
    out_loc = nc.dram_tensor("out_loc", [D, T], F32)
    out_red = nc.dram_tensor("out_red", [DS, T], F32)

    groups = [[g * Gc + i for i in range(Gc)] for g in range(n_groups)]

    def silu_into(pool, out_ap, in_ap, bias_ap=None):
        if act_impl == "hw":
            if bias_ap is None:
                nc.scalar.activation(out_ap, in_ap, AF.Silu)
            else:
                nc.scalar.activation(out_ap, in_ap, AF.Silu, bias=bias_ap)
        else:  # sim fallback: silu(v) = v * sigmoid(v), v = in + bias
            P_, F_ = in_ap.shape[0], in_ap.shape[1]
            v = pool.tile([P_, F_], F32, tag="siluv", name="siluv")
            nc.vector.tensor_scalar(v[:], in_ap,
                                    bias_ap if bias_ap is not None else 0.0,
                                    None, OP.add)
            sg = pool.tile([P_, F_], F32, tag="silusg", name="silusg")
            nc.scalar.activation(sg[:], v[:], AF.Sigmoid)
            nc.vector.tensor_tensor(out_ap, v[:], sg[:], OP.mult)

    def softplus_into(pool, out_ap, in_ap, bias_ap):
        # softplus(v) = ln(1 + exp(v)); the walrus lower_act pass has no
        # table set for the Softplus enum, so decompose (Exp and Ln share
        # the natural_log_exp_and_others set).
        P_, F_ = in_ap.shape[0], in_ap.shape[1]
        e = pool.tile([P_, F_], F32, tag="spe", name="spe", bufs=2)
        nc.scalar.activation(e[:], in_ap, AF.Exp, bias=bias_ap)
        nc.vector.tensor_scalar(e[:], e[:], 1.0, None, OP.add)
        nc.scalar.activation(out_ap, e[:], AF.Ln)

    with TileContext(nc) as tc:
      for _rep in range(reps):
        with tc.tile_pool(name="persist", bufs=1) as pp:
            # ---------- small persistent params ----------
            xp_sb = pp.tile([128, M * XD], F32)
            nc.sync.dma_start(out=xp_sb[:], in_=xprojT.ap().rearrange("(m p) e -> p m e", p=128))
            dtp_sb = pp.tile([R, E_c], F32)
            nc.sync.dma_start(out=dtp_sb[:], in_=dtprojT[:, :])
            cw_sb = pp.tile([128, M * DCONV], F32)
            nc.sync.dma_start(out=cw_sb[:], in_=conv_w[:, :])
            cb_sb = pp.tile([128, M], F32)
            nc.sync.dma_start(out=cb_sb[:], in_=conv_b[:, :])
            dtb_sb = pp.tile([128, M], F32)
            nc.sync.dma_start(out=dtb_sb[:], in_=dtb[:, :])
            dp_sb = pp.tile([128, M], F32)
            nc.sync.dma_start(out=dp_sb[:], in_=Dp[:, :])
            al_sb = pp.tile([128, G], F32)
            nc.sync.dma_start(out=al_sb[:], in_=Alane[:, :])
            sb_sb = pp.tile([128, 16 * 128], F32)
            nc.sync.dma_start(out=sb_sb[:], in_=sel_bcast[:, :])
            spl_sb = pp.tile([128, 16 * 128], F32)
            nc.sync.dma_start(out=spl_sb[:], in_=sel_place[:, :])
            sr_sb = pp.tile([N, 128], F32)
            nc.sync.dma_start(out=sr_sb[:], in_=sel_rep16[:, :])

            xc_sb = [pp.tile([128, T], F32, tag=f"xc{m}", name=f"xc{m}") for m in range(M)]
            dt_sb = [pp.tile([128, T], F32, tag=f"dt{m}", name=f"dt{m}") for m in range(M)]

            # ---------- phase A: in_proj + fused conv ----------
            # xi is consumed chunk-by-chunk through a rolling [128, TC+3]
            # window per m; silu(z) streams to DRAM (s_dram).
            with (
                tc.tile_pool(name="pA", bufs=1) as pa,
                tc.tile_pool(name="pAw", bufs=2) as paw,
                tc.tile_pool(name="pAx", bufs=K + 2) as pax,
                tc.tile_pool(name="psA", bufs=4, space="PSUM") as psA,
            ):
                wxz = pa.tile([128, K * 2 * E_c], F32)
                nc.sync.dma_start(
                    out=wxz[:], in_=w_xzT.ap().rearrange("(k p) e -> p k e", p=128))
                win = [pa.tile([128, TC + DCONV - 1], F32, tag=f"win{m}", name=f"win{m}")
                       for m in range(M)]
                for m in range(M):
                    nc.vector.memset(win[m][:, 0:DCONV - 1], 0)

                for t in range(NT):
                    xtc = []
                    for k in range(K):
                        xk = pax.tile([128, TC], F32, tag="xt", name="xt")
                        nc.sync.dma_start(
                            out=xk[:], in_=xT[k * 128:(k + 1) * 128, t * TC:(t + 1) * TC])
                        xtc.append(xk)
                    for m in range(2 * M):
                        ps = psA.tile([128, TC], F32, tag="a", name="ps_a")
                        for k in range(K):
                            nc.tensor.matmul(
                                ps[:],
                                wxz[:, k * 2 * E_c + m * 128: k * 2 * E_c + (m + 1) * 128],
                                xtc[k][:],
                                start=(k == 0), stop=(k == K - 1),
                            )
                        if m < M:  # xi chunk into rolling window
                            nc.scalar.copy(
                                out=win[m][:, DCONV - 1:DCONV - 1 + TC], in_=ps[:])
                            # conv on this chunk
                            acc = paw.tile([128, TC], F32, tag="conv", name="conv_acc")
                            nc.vector.tensor_scalar(
                                acc[:], win[m][:, 0:TC],
                                cw_sb[:, m * DCONV: m * DCONV + 1], None, OP.mult)
                            for kk in range(1, DCONV):
                                accn = paw.tile([128, TC], F32, tag="conv", name="conv_accn")
                                nc.vector.scalar_tensor_tensor(
                                    out=accn[:], in0=win[m][:, kk:kk + TC],
                                    scalar=cw_sb[:, m * DCONV + kk: m * DCONV + kk + 1],
                                    in1=acc[:], op0=OP.mult, op1=OP.add)
                                acc = accn
                            silu_into(paw, xc_sb[m][:, t * TC:(t + 1) * TC], acc[:],
                                      cb_sb[:, m:m + 1])
                            if t < NT - 1:  # roll last 3 cols to the front
                                nc.vector.tensor_copy(
                                    out=win[m][:, 0:DCONV - 1],
                                    in_=win[m][:, TC:TC + DCONV - 1])
                        else:      # z part -> silu -> DRAM
                            sz = paw.tile([128, TC], F32, tag="sz", name="sz")
                            silu_into(paw, sz[:], ps[:])
                            nc.sync.dma_start(
                                out=s_dram[(m - M) * 128:(m - M + 1) * 128,
                                           t * TC:(t + 1) * TC],
                                in_=sz[:])

            # ---------- x_proj partials + all-reduce ----------
            with (
                tc.tile_pool(name="pXd", bufs=1) as pxd,
                tc.tile_pool(name="psB", bufs=2, space="PSUM") as psB,
            ):
                xdbl_sb = pxd.tile([XD, T], F32)
                for t in range(NT):
                    ps = psB.tile([XD, TC], F32, tag="b", name="ps_b")
                    for m in range(M):
                        nc.tensor.matmul(
                            ps[:],
                            xp_sb[:, m * XD:(m + 1) * XD],
                            xc_sb[m][:, t * TC:(t + 1) * TC],
                            start=(m == 0), stop=(m == M - 1),
                        )
                    nc.scalar.copy(out=xdbl_sb[:, t * TC:(t + 1) * TC], in_=ps[:])
                nc.sync.dma_start(out=xdbl_loc[:, :], in_=xdbl_sb[:])

            if dbg_no_collectives:
                nc.gpsimd.dma_start(out=xdbl_red[:, :], in_=xdbl_loc[:, :])
            else:
                nc.gpsimd.collective_compute(
                    "AllReduce", OP.add, replica_groups=groups,
                    ins=[xdbl_loc.ap().opt()], outs=[xdbl_red.ap().opt()])

            # ---------- phase C/D: dt_proj -> dt ; B_rep/C_rep ----------
            with tc.tile_pool(name="pBC", bufs=1) as pbc:
                brep_sb = pbc.tile([128, T], F32)
                crep_sb = pbc.tile([128, T], F32)
                with (
                    tc.tile_pool(name="pCD", bufs=1) as pcd,
                    tc.tile_pool(name="psC", bufs=2, space="PSUM") as psC,
                ):
                    dtlo_sb = pcd.tile([R, T], F32)
                    b_sb = pcd.tile([N, T], F32)
                    c_sb = pcd.tile([N, T], F32)
                    nc.sync.dma_start(out=dtlo_sb[:], in_=xdbl_red[0:R, :])
                    nc.sync.dma_start(out=b_sb[:], in_=xdbl_red[R:R + N, :])
                    nc.sync.dma_start(out=c_sb[:], in_=xdbl_red[R + N:XD, :])
                    for m in range(M):
                        for t in range(NT):
                            ps = psC.tile([128, TC], F32, tag='c', name='ps_c')
                            nc.tensor.matmul(
                                ps[:], dtp_sb[:, m * 128:(m + 1) * 128],
                                dtlo_sb[:, t * TC:(t + 1) * TC],
                                start=True, stop=True)
                            softplus_into(pcd, dt_sb[m][:, t * TC:(t + 1) * TC], ps[:],
                                          dtb_sb[:, m:m + 1])
                    for t in range(NT):
                        psb_ = psC.tile([128, TC], F32, tag='c', name='psb_c')
                        nc.tensor.matmul(psb_[:], sr_sb[:],
                                         b_sb[:, t * TC:(t + 1) * TC],
                                         start=True, stop=True)
                        nc.scalar.copy(out=brep_sb[:, t * TC:(t + 1) * TC], in_=psb_[:])
                        psc_ = psC.tile([128, TC], F32, tag='c', name='psc_c')
                        nc.tensor.matmul(psc_[:], sr_sb[:],
                                         c_sb[:, t * TC:(t + 1) * TC],
                                         start=True, stop=True)
                        nc.scalar.copy(out=crep_sb[:, t * TC:(t + 1) * TC], in_=psc_[:])

                # ---------- phase E: scan ----------
                z_eng = getattr(nc, z_engine)
                gate_eng = getattr(nc, gate_engine)
                yd_list = []
                pyd_cm = tc.tile_pool(name="pyd", bufs=M)
                pyd = pyd_cm.__enter__()
                with (
                    tc.tile_pool(name="pScan", bufs=2) as psc_sb,
                    tc.tile_pool(name="psDT", bufs=2, space="PSUM") as psDT,
                    tc.tile_pool(name="psU", bufs=2, space="PSUM") as psU,
                    tc.tile_pool(name="psY", bufs=1, space="PSUM") as psY,
                ):
                    for m in range(M):
                        y_ps = psY.tile([128, T], F32, tag="y", name="y_ps")
                        u_m = psc_sb.tile([128, T], F32, tag="u", name="u_m")
                        nc.vector.tensor_tensor(u_m[:], dt_sb[m][:], xc_sb[m][:], OP.mult)
                        for jj in range(16):
                            g = m * 16 + jj
                            a_t = psc_sb.tile([128, T], F32, tag="a", name="a_t")
                            dbx_t = psc_sb.tile([128, T], F32, tag="dbx", name="dbx_t")
                            for t in range(NT):
                                tcs = slice(t * TC, (t + 1) * TC)
                                dtr = psDT.tile([128, TC], F32, tag="dt", name="dtr")
                                nc.tensor.matmul(dtr[:], sb_sb[:, jj * 128:(jj + 1) * 128],
                                                 dt_sb[m][:, tcs], start=True, stop=True)
                                nc.scalar.activation(a_t[:, tcs], dtr[:], AF.Exp,
                                                     scale=al_sb[:, g:g + 1])
                                ur = psU.tile([128, TC], F32, tag="u", name="ur")
                                nc.tensor.matmul(ur[:], sb_sb[:, jj * 128:(jj + 1) * 128],
                                                 u_m[:, tcs], start=True, stop=True)
                                nc.vector.tensor_tensor(dbx_t[:, tcs], ur[:],
                                                        brep_sb[:, tcs], OP.mult)
                            # h over a_t (in-place), z over h (in-place)
                            if dbg_scan_as_mult:
                                nc.vector.tensor_tensor(a_t[:], a_t[:], dbx_t[:],
                                                        OP.mult)
                            else:
                                nc.vector.tensor_tensor_scan(a_t[:], a_t[:], dbx_t[:],
                                                             0.0, OP.mult, OP.add)
                            z_eng.tensor_tensor(a_t[:], a_t[:], crep_sb[:], OP.mult)
                            for t in range(NT):
                                tcs = slice(t * TC, (t + 1) * TC)
                                nc.tensor.matmul(y_ps[:, tcs],
                                                 spl_sb[:, jj * 128:(jj + 1) * 128],
                                                 a_t[:, tcs],
                                                 start=(jj == 0), stop=(jj == 15))
                        s_m = psc_sb.tile([128, T], F32, tag="s", name="s_m")
                        nc.sync.dma_start(out=s_m[:],
                                          in_=s_dram[m * 128:(m + 1) * 128, :])
                        yd = pyd.tile([128, T], F32, tag="yd", name="yd")
                        nc.vector.scalar_tensor_tensor(
                            out=yd[:], in0=xc_sb[m][:], scalar=dp_sb[:, m:m + 1],
                            in1=y_ps[:], op0=OP.mult, op1=OP.add)
                        gate_eng.tensor_tensor(yd[:], yd[:], s_m[:], OP.mult)
                        yd_list.append(yd)

                # ---------- phase F: out_proj partials + reduce-scatter ----------
                with (
                    tc.tile_pool(name="pF", bufs=1) as pf,
                    tc.tile_pool(name="pFe", bufs=3) as pfe,
                    tc.tile_pool(name="psF", bufs=2, space="PSUM") as psF,
                ):
                    opT_sb = pf.tile([128, M * D], F32)
                    nc.sync.dma_start(
                        out=opT_sb[:],
                        in_=opT.ap().rearrange("(m p) d -> p m d", p=128))
                    for mo in range(MO):
                        for t in range(NT):
                            tcs = slice(t * TC, (t + 1) * TC)
                            ps = psF.tile([128, TC], F32, tag="f", name="ps_f")
                            for k in range(M):
                                nc.tensor.matmul(
                                    ps[:],
                                    opT_sb[:, k * D + mo * 128: k * D + (mo + 1) * 128],
                                    yd_list[k][:, tcs],
                                    start=(k == 0), stop=(k == M - 1))
                            ev = pfe.tile([128, TC], F32, tag="ev", name="ev")
                            nc.scalar.copy(out=ev[:], in_=ps[:])
                            nc.sync.dma_start(
                                out=out_loc[mo * 128:(mo + 1) * 128, tcs], in_=ev[:])
                pyd_cm.__exit__(None, None, None)

            if dbg_no_collectives:
                nc.gpsimd.dma_start(out=out_red[:, :], in_=out_loc[0:DS, :])
            else:
                nc.gpsimd.collective_compute(
                    "ReduceScatter", OP.add, replica_groups=groups,
                    ins=[out_loc.ap().opt()], outs=[out_red.ap().opt()])
            nc.gpsimd.dma_start(out=out_c[:, :], in_=out_red[:, :])

    return nc


# ---------------- walrus wait-limit workaround ----------------
def split_excess_waits(nc, max_waits=1):
    """This walrus build rejects instructions with >1 sync-wait command.
    Move excess waits onto NoOps inserted before the instruction on the
    same engine (same-engine stream order keeps semantics identical)."""
    import concourse.mybir as mybir
    n_split = 0
    for fn in nc.m.functions:
        for blk in fn.blocks:
            insts = list(blk.instructions)
            out = []
            changed = False
            for inst in insts:
                si = getattr(inst, "sync_info", None)
                waits = list(si.on_wait) if (si is not None and si.on_wait) else []
                if len(waits) > max_waits:
                    chunks = [waits[i:i + max_waits]
                              for i in range(0, len(waits), max_waits)]
                    for j, ch in enumerate(chunks[:-1]):
                        nop = mybir.InstNoOp(
                            name=f"{inst.name}-wsplit{j}", ins=[], outs=[])
                        nop.engine = inst.engine
                        nop.sync_info = mybir.SyncInfo(on_wait=ch, on_update=[])
                        out.append(nop)
                    si.on_wait = chunks[-1]
                    n_split += 1
                    changed = True
                out.append(inst)
            if changed:
                blk.instructions.clear()
                for i in out:
                    blk.instructions.append(i)
    return n_split


# ---------------- host-side prep ----------------
def _make_selectors(N=16):
    sel_bcast = np.zeros((128, 16 * 128), np.float32)
    sel_place = np.zeros((128, 16 * 128), np.float32)
    for j in range(16):
        for l in range(128):
            e = j * 8 + l // 16
            sel_bcast[e, j * 128 + l] = 1.0
            sel_place[l, j * 128 + e] = 1.0
    sel_rep16 = np.zeros((N, 128), np.float32)
    for l in range(128):
        sel_rep16[l % N, l] = 1.0
    return sel_bcast, sel_place, sel_rep16


def make_core_inputs(inputs, D, E, E_c, T, N=16, R=64, DCONV=4, n_groups=2):
    Gc = E // E_c
    M = E_c // 128
    G = E_c // 8
    x = np.asarray(inputs["x"], np.float32)
    in_proj_w = np.asarray(inputs["in_proj_w"], np.float32)
    conv_w = np.asarray(inputs["conv_w"], np.float32)
    conv_b = np.asarray(inputs["conv_b"], np.float32)
    x_proj_w = np.asarray(inputs["x_proj_w"], np.float32)
    dt_proj_w = np.asarray(inputs["dt_proj_w"], np.float32)
    dt_proj_b = np.asarray(inputs["dt_proj_b"], np.float32)
    A_log = np.asarray(inputs["A_log"], np.float32)
    Dp = np.asarray(inputs["Dp"], np.float32)
    out_proj_w = np.asarray(inputs["out_proj_w"], np.float32)

    A = -np.exp(A_log)
    opT_full = np.ascontiguousarray(out_proj_w.T)
    sel_bcast, sel_place, sel_rep16 = _make_selectors(N)

    maps = []
    for c in range(n_groups * Gc):
        b, ec = c // Gc, c % Gc
        es = ec * E_c
        w_xz = np.concatenate(
            [in_proj_w[es:es + E_c], in_proj_w[E + es:E + es + E_c]], axis=0)
        eidx = np.arange(E_c) + es
        lane_e = np.arange(128) // 16
        lane_n = np.arange(128) % 16
        g_idx = np.arange(G)
        Alane = A[(es + g_idx[None, :] * 8 + lane_e[:, None]), lane_n[:, None]]
        pm = np.arange(128)[:, None] + 128 * np.arange(M)[None, :]
        maps.append({
            "xT": np.ascontiguousarray(x[b].T),
            "w_xzT": np.ascontiguousarray(w_xz.T),
            "xprojT": np.ascontiguousarray(x_proj_w[:, es:es + E_c].T),
            "dtprojT": np.ascontiguousarray(dt_proj_w[es:es + E_c].T),
            "opT": np.ascontiguousarray(opT_full[es:es + E_c]),
            "conv_w": np.ascontiguousarray(
                conv_w[eidx].reshape(M, 128, DCONV).transpose(1, 0, 2)
                .reshape(128, M * DCONV)),
            "conv_b": np.ascontiguousarray(conv_b[eidx[pm]]),
            "dtb": np.ascontiguousarray(dt_proj_b[eidx[pm]]),
            "Dp": np.ascontiguousarray(Dp[eidx[pm]]),
            "Alane": np.ascontiguousarray(Alane.astype(np.float32)),
            "sel_bcast": sel_bcast,
            "sel_place": sel_place,
            "sel_rep16": sel_rep16,
        })
    return maps


def assemble_output(core_outs, D, E, E_c, T, B, n_groups=2):
    Gc = E // E_c
    DS = D // Gc
    out = np.empty((B, T, D), np.float32)
    for c in range(n_groups * Gc):
        b, ec = c // Gc, c % Gc
        out[b, :, ec * DS:(ec + 1) * DS] = core_outs[c].T
    return out


# ---------------- cached PJRT runner ----------------
_RUNNER = None


class _Runner:
    """Builds the Bass module once and keeps a reusable jitted shard_map
    callable (mirrors concourse.bass2jax.run_bass_via_pjrt, but cached so
    repeated calls don't recompile)."""

    def __init__(self):
        import jax
        import jax.numpy as jnp  # noqa: F401
        from jax.experimental.shard_map import shard_map
        from jax.sharding import Mesh, PartitionSpec
        import concourse.mybir as mybir
        from concourse import bass2jax

        self.nc = build_mamba_nc(D_MODEL, D_INNER, E_C, T_SEQ, N=D_STATE,
                                 R=DT_RANK, DCONV=D_CONV, n_groups=N_GROUPS)
        split_excess_waits(self.nc)

        bass2jax.install_neuronx_cc_hook()
        nc = self.nc
        assert nc.dbg_addr is None
        partition_name = (nc.partition_id_tensor.name
                          if nc.partition_id_tensor else None)

        in_names, out_names, out_avals, zero_outs = [], [], [], []
        for alloc in nc.m.functions[0].allocations:
            if not isinstance(alloc, mybir.MemoryLocationSet):
                continue
            name = alloc.memorylocations[0].name
            if alloc.kind == "ExternalInput":
                if name != partition_name:
                    in_names.append(name)
            elif alloc.kind == "ExternalOutput":
                shape = tuple(alloc.tensor_shape)
                dtype = mybir.dt.np(alloc.dtype)
                out_names.append(name)
                out_avals.append(jax.core.ShapedArray(shape, dtype))
                zero_outs.append(np.zeros(shape, dtype))
        n_params = len(in_names)
        n_outs = len(out_avals)
        all_names = in_names + out_names
        if partition_name is not None:
            all_names = all_names + [partition_name]
        donate = tuple(range(n_params, n_params + n_outs))
        self.in_names = in_names
        self.out_names = out_names
        self.zero_outs = zero_outs
        self.n_cores = N_CORES

        def _body(*args):
            operands = list(args)
            if partition_name is not None:
                operands.append(bass2jax.partition_id_tensor())
            outs = bass2jax._bass_exec_p.bind(
                *operands,
                out_avals=tuple(out_avals),
                in_names=tuple(all_names),
                out_names=tuple(out_names),
                lowering_input_output_aliases=(),
                sim_require_finite=True,
                sim_require_nnan=True,
                nc=nc,
            )
            return tuple(outs)

        devices = jax.devices()[:N_CORES]
        assert len(devices) == N_CORES
        mesh = Mesh(np.asarray(devices), ("core",))
        in_specs = (PartitionSpec("core"),) * (n_params + n_outs)
        out_specs = (PartitionSpec("core"),) * n_outs
        self._fn = jax.jit(
            shard_map(_body, mesh=mesh, in_specs=in_specs,
                      out_specs=out_specs, check_rep=False),
            donate_argnums=donate, keep_unused=True)
        self._mesh = mesh
        self._jax = jax

    def put_inputs(self, in_maps):
        """Pre-stage concatenated inputs on the device mesh (axis-0 sharded).
        Returns a handle reusable across run_prestaged calls."""
        import jax
        from jax.sharding import NamedSharding, PartitionSpec
        sh = NamedSharding(self._mesh, PartitionSpec("core"))
        concat_in = [
            np.concatenate([np.asarray(in_maps[c][n]) for c in range(self.n_cores)],
                           axis=0)
            for n in self.in_names
        ]
        arrs = [jax.device_put(a, sh) for a in concat_in]
        for a in arrs:
            a.block_until_ready()
        return arrs

    def fresh_zero_outs(self):
        import jax
        from jax.sharding import NamedSharding, PartitionSpec
        sh = NamedSharding(self._mesh, PartitionSpec("core"))
        zs = [jax.device_put(np.concatenate([z] * self.n_cores, axis=0), sh)
              for z in self.zero_outs]
        for z in zs:
            z.block_until_ready()
        return zs

    def run_prestaged(self, dev_in, dev_zeros):
        """Execute with device-resident args; blocks until done; leaves
        outputs on device. Returns the raw jax output tuple."""
        outs = self._fn(*dev_in, *dev_zeros)
        for o in outs:
            o.block_until_ready()
        return outs

    def __call__(self, in_maps):
        concat_in = [
            np.concatenate([np.asarray(in_maps[c][n]) for c in range(self.n_cores)],
                           axis=0)
            for n in self.in_names
        ]
        concat_zero = [
            np.concatenate([z] * self.n_cores, axis=0) for z in self.zero_outs
        ]
        outs = self._fn(*concat_in, *concat_zero)
        results = []
        for c in range(self.n_cores):
            r = {}
            for i, n in enumerate(self.out_names):
                arr = np.asarray(outs[i])
                per = arr.shape[0] // self.n_cores
                r[n] = arr[c * per:(c + 1) * per]
            results.append(r)
        return results


def _get_runner():
    global _RUNNER
    if _RUNNER is None:
        _RUNNER = _Runner()
    return _RUNNER


def kernel(**inputs):
    runner = _get_runner()
    maps = make_core_inputs(inputs, D_MODEL, D_INNER, E_C, T_SEQ, N=D_STATE,
                            R=DT_RANK, DCONV=D_CONV, n_groups=N_GROUPS)
    outs = runner(maps)
    return assemble_output([o["out_c"] for o in outs], D_MODEL, D_INNER, E_C,
                           T_SEQ, B_SZ, n_groups=N_GROUPS)
